# revision 26
# baseline (speedup 1.0000x reference)
"""BitLinear kernel for Trainium2 (8 NeuronCores, tensor-parallel).

Computes: out = x @ (sign(w) * mean(|w|, axis=1, keepdims=True)).T
  x      : [4, 2048, 4096] f32
  weight : [4096, 4096] f32
  out    : [4, 2048, 4096] f32

Strategy (per sharding hint): shard weight rows (out features) 8-way.
Hybrid-precision contraction: the first K16 k-tiles run in bf16
(128-deep per matmul), the remaining 2*F8P k-tiles run as e4m3 fp8
DoubleRow pairs (256-deep per matmul at ~2x the bf16 PE rate).  Signs
are exactly representable in both dtypes, so the only extra error is
the e4m3 quantization of the x slice routed through fp8:
rel_l2 ~= 0.0265 * sqrt(2*F8P/32), ~1.75e-2 at F8P=7 (gate: 2e-2).

Each core:
  - receives x.T pre-tiled on host: pair 0 as per-k-tile 256KB chunks
    (xTp16 bf16 / x8p e4m3 DoubleRow slot layout) for the HBM-paced
    startup, pairs 1-7 partition-major as one 4.5MB + one 1.75MB DMA
    each (xTr / x8r); its weight shard k-tile-pair transposed (wTt,
    the sign operand source) and the per-row scales s[n] = mean|w|
    precomputed on host (sc, sharded with their rows per the hint).
  - binarizes on device: Sign activation -> bf16 S_all for the bf16
    k-tiles and fp8e4 S8_all for the DoubleRow pairs (+-1 exact).
  - matmuls: per 512-token block and 128-feature n-tile, K16 bf16
    matmuls then F8P DoubleRow matmuls accumulate one PSUM bank; the
    f32 per-feature scale is applied while evicting PSUM -> SBUF;
    stores write the feature-major shard outT [512, 8192].
  - a 9-matmul junk pre-warm during the NEFF preamble gets the HAM
    clock gate to K=8/8 before the first real matmul.
Host gathers the 8 outT shards -> [4096, 8192] -> transpose -> out.
"""

import os
from contextlib import ExitStack

import numpy as np
import ml_dtypes

import concourse.bass as bass
import concourse.mybir as mybir
import concourse.tile as tile
from concourse import bacc, bass_utils

P = 128                 # SBUF partitions / PE array dim
D_IN = 4096             # contraction dim (in features)
D_OUT = 4096            # out features
M_TOT = 8192            # tokens (4*2048)
N_CORES = 8
N_SHARD = D_OUT // N_CORES      # 512 out features per core
K_TILES = D_IN // P             # 32
M_BLK = 512                     # moving free dim per matmul
M_BLKS = M_TOT // M_BLK         # 16
M_PAIRS = M_BLKS // 2           # 8 (x is loaded in block pairs)
N_TILES = N_SHARD // P          # 4

# Hybrid precision split: k-tiles [0, K16) in bf16, [K16, 32) as fp8
# DoubleRow pairs.  K16 must be even (wTt pair layout).  K16=16 measures
# rel_l2 = 1.873e-2 (deterministic; gate 2e-2); K16=18 = 1.753e-2.
K16 = int(os.environ.get("BITLIN_K16", "16"))
F8P = (K_TILES - K16) // 2
assert K16 % 2 == 0 and K16 + 2 * F8P == K_TILES

PAIR_W = 2 * M_BLK      # 1024 tokens per x block pair

_CACHE = {}
LAST_RESULTS = None  # BassKernelResults of the most recent run (for test harness)


def _install_ntff_hook():
    """Register the ctypes NTFF profiling hook under antenv.axon_hooks so
    run_bass_kernel_spmd(trace=True) can capture device profiles under axon.
    No-op if already present or the .so lacks the symbols."""
    import contextlib
    import ctypes
    import sys
    import types

    try:
        from antenv.axon_hooks import get_axon_ntff_profile_hook  # noqa: F401

        return True
    except ImportError:
        pass

    so_path = "/opt/axon/libaxon_pjrt.so"
    if not os.path.exists(so_path):
        return False
    lib = ctypes.CDLL(so_path)
    if not hasattr(lib, "axon_start_nrt_profile"):
        return False
    lib.axon_start_nrt_profile.argtypes = [
        ctypes.POINTER(ctypes.c_int64),
        ctypes.c_size_t,
    ]
    lib.axon_start_nrt_profile.restype = ctypes.c_int64
    lib.axon_stop_nrt_profile.argtypes = [ctypes.c_char_p]
    lib.axon_stop_nrt_profile.restype = ctypes.c_int64

    @contextlib.contextmanager
    def _hook(output_dir, device_ids):
        import jax

        jax.devices()
        if device_ids:
            ids = (ctypes.c_int64 * len(device_ids))(*device_ids)
            rc = lib.axon_start_nrt_profile(ids, len(device_ids))
        else:
            rc = lib.axon_start_nrt_profile(None, 0)
        if rc != 0:
            raise RuntimeError(f"axon_start_nrt_profile rc={rc}")
        try:
            yield
        finally:
            n = lib.axon_stop_nrt_profile(str(output_dir).encode())
            print(f"ntff profile: {n} file(s) written to {output_dir}")

    mod = types.ModuleType("antenv.axon_hooks")
    _state = {"hook": _hook}
    mod.set_axon_ntff_profile_hook = lambda h: _state.__setitem__("hook", h)
    mod.get_axon_ntff_profile_hook = lambda: _state["hook"]
    sys.modules["antenv.axon_hooks"] = mod
    import antenv

    antenv.axon_hooks = mod

    # artifact upload reaches for a cloud bucket that isn't available here
    bass_utils.upload_artifacts = lambda tmpdir: f"local:{tmpdir}"
    return True


def _build_nc():
    nc = bacc.Bacc(
        "TRN2", target_bir_lowering=False, debug=False, num_devices=N_CORES,
        enable_partition_id=False,
    )
    # Pair 0's x arrives per-k-tile (xTp16[j, p, m] = x.T[j*128+p, m],
    # 256KB chunks with 2KB-per-partition packets) so the HBM-paced startup
    # can feed the j-outer warmup chunk by chunk.  Pairs 1..7 arrive
    # partition-major (xTr[q-1, p, j*1024+m]) as ONE 4.5MB DMA per pair:
    # fewer chunk semaphores means ~25 fewer semaphore-carrying matmuls per
    # pair (each costs ~215ns of PE dispatch even when already satisfied).
    xTp16 = nc.dram_tensor(
        "xTp16", [K16, P, PAIR_W], mybir.dt.bfloat16, kind="ExternalInput",
    )
    xTr = nc.dram_tensor(
        "xTr", [M_PAIRS - 1, P, K16 * PAIR_W], mybir.dt.bfloat16,
        kind="ExternalInput",
    )
    # fp8 x DoubleRow pairs, same split: pair 0 per k-pair chunk
    # (x8p[jf, p, h*1024 + m], e4m3), pairs 1..7 as one DMA each.
    x8p = nc.dram_tensor(
        "x8p", [max(F8P, 1), P, 2 * PAIR_W], mybir.dt.float8e4,
        kind="ExternalInput",
    )
    x8r = nc.dram_tensor(
        "x8r", [M_PAIRS - 1, P, 2 * F8P * PAIR_W], mybir.dt.float8e4,
        kind="ExternalInput",
    )
    # Per-out-feature scales s[n] = mean_k |w[n, k]|, computed on host during
    # input prep (the sharding hint ships per-row scales with their rows) and
    # laid out per n-tile as a [128, 1] per-partition column (column i =
    # n-tile i).  Loading 2KB here instead of reducing 4MB of |w| on device
    # keeps the scale off the first eviction's critical path.
    sc = nc.dram_tensor("sc", [P, N_TILES], mybir.dt.float32, kind="ExternalInput")
    # w.T pre-tiled in k-tile pairs: wTt[jj, p, h*512+n] = w.T[(2*jj+h)*128+p, n]
    # so each DMA has 2KB-per-partition packets.
    wTt = nc.dram_tensor(
        "wTt", [K_TILES // 2, P, 2 * N_SHARD], mybir.dt.bfloat16,
        kind="ExternalInput",
    )
    # Output shard stored as bf16 (host upcasts to f32): halves the store
    # DMA bytes and the SBUF traffic that collides with the matmul stream
    # (~2 of 4 per-block stores each cost a ~430ns matmul slip at f32).
    # Adds only ~0.17% L2 in quadrature: 1.873e-2 -> ~1.881e-2 measured gate
    # margin stays ~6%.
    outT = nc.dram_tensor(
        "outT", [N_SHARD, M_TOT], mybir.dt.bfloat16, kind="ExternalOutput"
    )

    with tile.TileContext(nc) as tc, ExitStack() as ctx:
        spool = ctx.enter_context(tc.tile_pool(name="scales", bufs=1))
        wtpool = ctx.enter_context(tc.tile_pool(name="wtrans", bufs=6))
        sgpool = ctx.enter_context(tc.tile_pool(name="sign", bufs=1))
        sg8pool = ctx.enter_context(tc.tile_pool(name="sign8", bufs=1))
        xpool = ctx.enter_context(tc.tile_pool(name="xpair", bufs=2))
        x8pool = ctx.enter_context(tc.tile_pool(name="x8pair", bufs=2))
        opool = ctx.enter_context(tc.tile_pool(name="oblk", bufs=6))
        ppool = ctx.enter_context(tc.tile_pool(name="psum", bufs=8, space="PSUM"))

        # PE pre-warm: a few junk matmuls issued during the NEFF preamble
        # dead time keep the PE busy from ~7.4us so the HAM clock gate
        # reaches K=8/8 (2.4 GHz) before the first real matmul; otherwise
        # the first ~13 real matmuls run at 1.2 GHz (427ns cadence).
        dmy = spool.tile([P, 5 * P], mybir.dt.bfloat16, name="dmy")
        nc.gpsimd.memset(dmy[:], 0.0)
        pwarm = ppool.tile([P, M_BLK], mybir.dt.float32, tag="ps", name="ps_warm")
        for _ in range(9):
            nc.tensor.matmul(
                pwarm[:], dmy[:, 0:P], dmy[:, P : 5 * P], start=True, stop=True
            )

        # Queue assignment: sync = x loads + sign-weight loads (chained in
        # emission order so the FIFO queue is deterministic); scalar =
        # scales load, then output stores (which must wait on evictions
        # and would stall x loads).
        prev_sync_dma = [None]

        def sync_load(dst, src):
            dma = nc.sync.dma_start(dst, src)
            if prev_sync_dma[0] is not None:
                # add_dep_helper(waiter, dependency): this load is ordered
                # after the previous one on the sync queue.
                tile.add_dep_helper(
                    dma.ins, prev_sync_dma[0].ins, sync=False,
                    reason="sync DMA queue emission order",
                )
            prev_sync_dma[0] = dma
            return dma

        def issue_x_pair(q):
            # bf16 tile [P, K16, 1024]; fp8 tile [P, 2*F8P, 1024] where the
            # middle dim (2*jf + h) is the DoubleRow slot dim per pair.
            # One DMA per tile (partition-major source layout).
            xt = xpool.tile([P, K16, PAIR_W], mybir.dt.bfloat16, tag="xpair")
            x8t = x8pool.tile(
                [P, 2 * F8P, PAIR_W], mybir.dt.float8e4, tag="x8pair"
            )
            sync_load(xt[:, :, :], xTr[q - 1, :, :])
            sync_load(x8t[:, :, :], x8r[q - 1, :, :])
            return xt, x8t

        def mm16(pss, xt, b, ni, j):
            nc.tensor.matmul(
                pss[ni][:],
                S_all[:, j * N_SHARD + ni * P : j * N_SHARD + (ni + 1) * P],
                xt[:, j, b * M_BLK : (b + 1) * M_BLK],
                start=(j == 0),
                stop=False,
            )

        def mm8(pss, x8t, b, ni, jf):
            nc.tensor.matmul(
                pss[ni][:],
                S8_all[:, 2 * jf : 2 * jf + 2, ni * P : (ni + 1) * P],
                x8t[:, 2 * jf : 2 * jf + 2, b * M_BLK : (b + 1) * M_BLK],
                start=False,
                stop=(jf == F8P - 1),
                perf_mode=mybir.MatmulPerfMode.DoubleRow,
            )

        def evict_block(pss, mb):
            # Evictions alternate between the scalar and vector engines so
            # the per-block eviction chain (and the kernel tail) is half as
            # long. Stores ride the scalar queue; for the final block the
            # sync queue (drained of x loads by then) takes half the store
            # triggers so the tail isn't serialized on one engine.
            last = mb == M_BLKS - 1
            for ni in range(N_TILES):
                ot = opool.tile([P, M_BLK], mybir.dt.bfloat16, tag="ot", name="ot")
                dst = outT[ni * P : (ni + 1) * P, mb * M_BLK : (mb + 1) * M_BLK]
                if ni % 2 == 0:
                    nc.scalar.mul(ot[:], pss[ni][:], s_all[:, ni : ni + 1])
                else:
                    nc.vector.tensor_scalar_mul(
                        ot[:], pss[ni][:], s_all[:, ni : ni + 1]
                    )
                if last and ni % 2 == 1:
                    nc.sync.dma_start(dst, ot[:])
                else:
                    nc.scalar.dma_start(dst, ot[:])

        # Prologue: interleave sign-weight loads with the first x pair's
        # loads on the chained sync queue so the earliest matmuls are fed in
        # lockstep with minimal latency.
        S_all = sgpool.tile([P, K16 * N_SHARD], mybir.dt.bfloat16)
        S8_all = sg8pool.tile([P, 2 * F8P, N_SHARD], mybir.dt.float8e4)
        xt0 = xpool.tile([P, K16, PAIR_W], mybir.dt.bfloat16, tag="xpair")
        x8t0 = x8pool.tile([P, 2 * F8P, PAIR_W], mybir.dt.float8e4, tag="x8pair")
        # Zero bias for the Sign activations as a plain SBUF tile (a float
        # bias would pull in a const-AP DRAM load during the preamble), and a
        # dummy 1-column sign to hoist the ACT LUT table load off the
        # critical path of the first real sign.
        zbias = spool.tile([P, 1], mybir.dt.float32)
        nc.gpsimd.memset(zbias[:], 0.0)
        nc.scalar.activation(
            S_all[:, 0:1], zbias[:], mybir.ActivationFunctionType.Sign,
            bias=zbias[:],
        )
        # fp8 DoubleRow pair sign: straight into the e4m3 tile (the scalar
        # engine converts on output; +-1 is exact in e4m3).  These loads are
        # interleaved among the bf16 pairs below so they are not stuck at the
        # tail of the startup DMA chain (their signs feed the pair-0 fp8
        # section at ~47us; at the chain tail they arrived several us late
        # and stalled the PE long enough to re-throttle HAM).
        def load_sign_fp8(jf):
            wt_t = wtpool.tile([P, 2 * N_SHARD], mybir.dt.bfloat16)
            dma = sync_load(wt_t[:], wTt[K16 // 2 + jf, :, :])
            nc.scalar.activation(
                S8_all[:, 2 * jf : 2 * jf + 2, :],
                wt_t[:],
                mybir.ActivationFunctionType.Sign,
                bias=zbias[:],
            )
            return dma

        # Scales: one 2KB load on the (otherwise idle) scalar dma queue.
        s_all = spool.tile([P, N_TILES], mybir.dt.float32)
        nc.scalar.dma_start(s_all[:], sc[:, :])

        fp8_pending = list(range(F8P))
        for jj in range(K16 // 2):
            wt_t = wtpool.tile([P, 2 * N_SHARD], mybir.dt.bfloat16)
            if jj == 0:
                # Split the first load/sign so the very first matmul (k-tile
                # 0, n-tile 0) is unblocked by a 32KB load + 128-col sign
                # instead of the full 512KB/1024-col pair, and slot the
                # first moving tile's load between the two sign chunks so
                # matmul #0's operands land back to back.
                sync_load(wt_t[:, 0:P], wTt[0, :, 0:P])
                nc.scalar.activation(
                    S_all[:, 0:P], wt_t[:, 0:P],
                    mybir.ActivationFunctionType.Sign, bias=zbias[:],
                )
                sync_load(xt0[:, 0, 0:M_BLK], xTp16[0, :, 0:M_BLK])
                sync_load(wt_t[:, P:], wTt[0, :, P:])
                nc.scalar.activation(
                    S_all[:, P : 2 * N_SHARD], wt_t[:, P:],
                    mybir.ActivationFunctionType.Sign, bias=zbias[:],
                )
                sync_load(xt0[:, 0, M_BLK:PAIR_W], xTp16[0, :, M_BLK:])
                sync_load(xt0[:, 1, :], xTp16[1, :, :])
            else:
                sync_load(wt_t[:], wTt[jj, :, :])
                nc.scalar.activation(
                    S_all[:, 2 * jj * N_SHARD : (2 * jj + 2) * N_SHARD],
                    wt_t[:],
                    mybir.ActivationFunctionType.Sign,
                    bias=zbias[:],
                )
                for j in (2 * jj, 2 * jj + 1):
                    sync_load(xt0[:, j, :], xTp16[j, :, :])
                if fp8_pending:
                    load_sign_fp8(fp8_pending.pop(0))
        while fp8_pending:
            load_sign_fp8(fp8_pending.pop(0))
        for jf in range(F8P):
            sync_load(x8t0[:, 2 * jf : 2 * jf + 2, :], x8p[jf, :, :])

        # Main loop: out.T[n, m] = sum_k S[k, n] * xT[k, m], scaled by s[n].
        # Pair 0 is computed j-outer across BOTH blocks (8 PSUM banks) so the
        # PE keeps pace with the HBM-limited startup stream; later pairs run
        # block-at-a-time j-outer (4 banks ping-ponging with the previous
        # block's draining 4).
        for q in range(M_PAIRS):
            xt, x8t = (xt0, x8t0) if q == 0 else issue_x_pair(q)
            if q == 0:
                pss2 = [
                    [
                        ppool.tile(
                            [P, M_BLK], mybir.dt.float32, tag="ps",
                            name=f"ps_{b}_{ni}",
                        )
                        for ni in range(N_TILES)
                    ]
                    for b in range(2)
                ]
                for j in range(K16):
                    for b in range(2):
                        for ni in range(N_TILES):
                            mm16(pss2[b], xt, b, ni, j)
                # fp8 tail of the warmup runs b-outer so block 0's stop
                # matmuls + eviction land ~7us early, freeing its PSUM banks
                # before pair 1's first accumulation needs them.
                for b in range(2):
                    for jf in range(F8P):
                        for ni in range(N_TILES):
                            mm8(pss2[b], x8t, b, ni, jf)
                    evict_block(pss2[b], b)
            else:
                for b in range(2):
                    last_blk = q == M_PAIRS - 1 and b == 1
                    pss = [
                        ppool.tile(
                            [P, M_BLK], mybir.dt.float32, tag="ps", name=f"ps{ni}"
                        )
                        for ni in range(N_TILES)
                    ]
                    if last_blk:
                        # ni-outer for the final block: each n-tile's stop
                        # matmul lands early, so its eviction + store overlap
                        # the remaining matmuls instead of serializing after
                        # the last one.
                        for ni in range(N_TILES):
                            for j in range(K16):
                                mm16(pss, xt, b, ni, j)
                            for jf in range(F8P):
                                mm8(pss, x8t, b, ni, jf)
                    else:
                        for j in range(K16):
                            for ni in range(N_TILES):
                                mm16(pss, xt, b, ni, j)
                        for jf in range(F8P):
                            for ni in range(N_TILES):
                                mm8(pss, x8t, b, ni, jf)
                    evict_block(pss, 2 * q + b)

    nc.compile()
    return nc


def kernel(x, weight):
    global LAST_RESULTS
    nc = _CACHE.get("nc")
    if nc is None:
        nc = _CACHE["nc"] = _build_nc()

    x = np.asarray(x)
    weight = np.asarray(weight)
    orig_shape = x.shape

    # Host-side sharding/layout: xT pre-tiled (replicated): bf16 k-tiles
    # [0, K16), e4m3 DoubleRow pairs [K16, 32); weight shard in both layouts.
    xT = x.reshape(M_TOT, D_IN).T  # [D_IN, M_TOT] view
    xTk = xT.reshape(K_TILES, P, M_PAIRS, PAIR_W)  # [kt, p, q, m]
    xb = xTk[:K16].astype(ml_dtypes.bfloat16)       # [kt, p, q, m]
    xf = xTk[K16:].astype(ml_dtypes.float8_e4m3)    # [kt'=2jf+h, p, q, m]
    # Pair 0 per-chunk layouts; pairs 1..7 partition-major (one DMA each).
    xTp16 = np.ascontiguousarray(xb[:, :, 0, :])    # [j, p, 1024]
    xTr = np.ascontiguousarray(
        xb.transpose(2, 1, 0, 3)[1:].reshape(M_PAIRS - 1, P, K16 * PAIR_W)
    )
    x8p = np.ascontiguousarray(
        xf[:, :, 0, :]
        .reshape(F8P, 2, P, PAIR_W)
        .transpose(0, 2, 1, 3)
        .reshape(F8P, P, 2 * PAIR_W)
    )  # [jf, p, h*1024+m]
    x8r = np.ascontiguousarray(
        xf.transpose(2, 1, 0, 3)[1:].reshape(M_PAIRS - 1, P, 2 * F8P * PAIR_W)
    )
    wt_full = np.ascontiguousarray(weight.T)  # [D_IN, D_OUT] f32
    # Per-row scales, sharded with their rows (see sharding hint): sc[p, i] =
    # mean|w[shard_base + i*128 + p, :]| as an f32 [128, N_TILES] column set.
    scales = np.abs(weight).mean(axis=1, dtype=np.float64).astype(np.float32)
    in_maps = []
    for c in range(N_CORES):
        in_maps.append(
            {
                "xTp16": xTp16,
                "xTr": xTr,
                "x8p": x8p,
                "x8r": x8r,
                "sc": np.ascontiguousarray(
                    scales[c * N_SHARD : (c + 1) * N_SHARD]
                    .reshape(N_TILES, P)
                    .T
                ),
                "wTt": np.ascontiguousarray(
                    wt_full[:, c * N_SHARD : (c + 1) * N_SHARD]
                    .reshape(K_TILES // 2, 2, P, N_SHARD)
                    .transpose(0, 2, 1, 3)
                    .reshape(K_TILES // 2, P, 2 * N_SHARD)
                    .astype(ml_dtypes.bfloat16)
                ),
            }
        )

    trace = bool(int(os.environ.get("BITLIN_TRACE", "0")))
    if trace:
        trace = _install_ntff_hook()
        base = os.environ.get("BITLIN_TRACE_DIR") or None
        if base:
            import tempfile

            os.makedirs(base, exist_ok=True)
            tmpdir = tempfile.mkdtemp(dir=base)
        else:
            tmpdir = None
    else:
        tmpdir = None
    res = bass_utils.run_bass_kernel_spmd(
        nc, in_maps, core_ids=list(range(N_CORES)), trace=trace, tmpdir=tmpdir
    )
    LAST_RESULTS = res

    outT_full = np.concatenate(
        [np.asarray(res.results[c]["outT"]) for c in range(N_CORES)], axis=0
    )  # [D_OUT, M_TOT] bf16
    out = np.ascontiguousarray(outT_full.T).reshape(orig_shape).astype(np.float32)
    return out


# revision 28
# speedup vs baseline: 1.0032x; 1.0032x over previous
"""BitLinear kernel for Trainium2 (8 NeuronCores, tensor-parallel).

Computes: out = x @ (sign(w) * mean(|w|, axis=1, keepdims=True)).T
  x      : [4, 2048, 4096] f32
  weight : [4096, 4096] f32
  out    : [4, 2048, 4096] f32

Strategy (per sharding hint): shard weight rows (out features) 8-way.
Hybrid-precision contraction: the first K16 k-tiles run in bf16
(128-deep per matmul), the remaining 2*F8P k-tiles run as e4m3 fp8
DoubleRow pairs (256-deep per matmul at ~2x the bf16 PE rate).  Signs
are exactly representable in both dtypes, so the only extra error is
the e4m3 quantization of the x slice routed through fp8:
rel_l2 ~= 0.0265 * sqrt(2*F8P/32), ~1.75e-2 at F8P=7 (gate: 2e-2).

Each core:
  - receives x.T pre-tiled on host: pair 0 as per-k-tile 256KB chunks
    (xTp16 bf16 / x8p e4m3 DoubleRow slot layout) for the HBM-paced
    startup, pairs 1-7 partition-major as one 4.5MB + one 1.75MB DMA
    each (xTr / x8r); its weight shard k-tile-pair transposed (wTt,
    the sign operand source) and the per-row scales s[n] = mean|w|
    precomputed on host (sc, sharded with their rows per the hint).
  - binarizes on device: Sign activation -> bf16 S_all for the bf16
    k-tiles and fp8e4 S8_all for the DoubleRow pairs (+-1 exact).
  - matmuls: per 512-token block and 128-feature n-tile, K16 bf16
    matmuls then F8P DoubleRow matmuls accumulate one PSUM bank; the
    f32 per-feature scale is applied while evicting PSUM -> SBUF;
    stores write the feature-major shard outT [512, 8192].
  - a 9-matmul junk pre-warm during the NEFF preamble gets the HAM
    clock gate to K=8/8 before the first real matmul.
Host gathers the 8 outT shards -> [4096, 8192] -> transpose -> out.
"""

import os
from contextlib import ExitStack

import numpy as np
import ml_dtypes

import concourse.bass as bass
import concourse.mybir as mybir
import concourse.tile as tile
from concourse import bacc, bass_utils

P = 128                 # SBUF partitions / PE array dim
D_IN = 4096             # contraction dim (in features)
D_OUT = 4096            # out features
M_TOT = 8192            # tokens (4*2048)
N_CORES = 8
N_SHARD = D_OUT // N_CORES      # 512 out features per core
K_TILES = D_IN // P             # 32
M_BLK = 512                     # moving free dim per matmul
M_BLKS = M_TOT // M_BLK         # 16
M_PAIRS = M_BLKS // 2           # 8 (x is loaded in block pairs)
N_TILES = N_SHARD // P          # 4

# Hybrid precision split: k-tiles [0, K16) in bf16, [K16, 32) as fp8
# DoubleRow pairs.  K16 must be even (wTt pair layout).  K16=16 measures
# rel_l2 = 1.873e-2 (deterministic; gate 2e-2); K16=18 = 1.753e-2.
K16 = int(os.environ.get("BITLIN_K16", "16"))
F8P = (K_TILES - K16) // 2
assert K16 % 2 == 0 and K16 + 2 * F8P == K_TILES

PAIR_W = 2 * M_BLK      # 1024 tokens per x block pair

_CACHE = {}
LAST_RESULTS = None  # BassKernelResults of the most recent run (for test harness)


def _install_ntff_hook():
    """Register the ctypes NTFF profiling hook under antenv.axon_hooks so
    run_bass_kernel_spmd(trace=True) can capture device profiles under axon.
    No-op if already present or the .so lacks the symbols."""
    import contextlib
    import ctypes
    import sys
    import types

    try:
        from antenv.axon_hooks import get_axon_ntff_profile_hook  # noqa: F401

        return True
    except ImportError:
        pass

    so_path = "/opt/axon/libaxon_pjrt.so"
    if not os.path.exists(so_path):
        return False
    lib = ctypes.CDLL(so_path)
    if not hasattr(lib, "axon_start_nrt_profile"):
        return False
    lib.axon_start_nrt_profile.argtypes = [
        ctypes.POINTER(ctypes.c_int64),
        ctypes.c_size_t,
    ]
    lib.axon_start_nrt_profile.restype = ctypes.c_int64
    lib.axon_stop_nrt_profile.argtypes = [ctypes.c_char_p]
    lib.axon_stop_nrt_profile.restype = ctypes.c_int64

    @contextlib.contextmanager
    def _hook(output_dir, device_ids):
        import jax

        jax.devices()
        if device_ids:
            ids = (ctypes.c_int64 * len(device_ids))(*device_ids)
            rc = lib.axon_start_nrt_profile(ids, len(device_ids))
        else:
            rc = lib.axon_start_nrt_profile(None, 0)
        if rc != 0:
            raise RuntimeError(f"axon_start_nrt_profile rc={rc}")
        try:
            yield
        finally:
            n = lib.axon_stop_nrt_profile(str(output_dir).encode())
            print(f"ntff profile: {n} file(s) written to {output_dir}")

    mod = types.ModuleType("antenv.axon_hooks")
    _state = {"hook": _hook}
    mod.set_axon_ntff_profile_hook = lambda h: _state.__setitem__("hook", h)
    mod.get_axon_ntff_profile_hook = lambda: _state["hook"]
    sys.modules["antenv.axon_hooks"] = mod
    import antenv

    antenv.axon_hooks = mod

    # artifact upload reaches for a cloud bucket that isn't available here
    bass_utils.upload_artifacts = lambda tmpdir: f"local:{tmpdir}"
    return True


def _build_nc():
    nc = bacc.Bacc(
        "TRN2", target_bir_lowering=False, debug=False, num_devices=N_CORES,
        enable_partition_id=False,
    )
    # Pair 0's x arrives per-k-tile (xTp16[j, p, m] = x.T[j*128+p, m],
    # 256KB chunks with 2KB-per-partition packets) so the HBM-paced startup
    # can feed the j-outer warmup chunk by chunk.  Pairs 1..7 arrive
    # partition-major (xTr[q-1, p, j*1024+m]) as ONE 4.5MB DMA per pair:
    # fewer chunk semaphores means ~25 fewer semaphore-carrying matmuls per
    # pair (each costs ~215ns of PE dispatch even when already satisfied).
    xTp16 = nc.dram_tensor(
        "xTp16", [K16, P, PAIR_W], mybir.dt.bfloat16, kind="ExternalInput",
    )
    xTr = nc.dram_tensor(
        "xTr", [M_PAIRS - 1, P, K16 * PAIR_W], mybir.dt.bfloat16,
        kind="ExternalInput",
    )
    # fp8 x DoubleRow pairs, same split: pair 0 per k-pair chunk
    # (x8p[jf, p, h*1024 + m], e4m3), pairs 1..7 as one DMA each.
    x8p = nc.dram_tensor(
        "x8p", [max(F8P, 1), P, 2 * PAIR_W], mybir.dt.float8e4,
        kind="ExternalInput",
    )
    x8r = nc.dram_tensor(
        "x8r", [M_PAIRS - 1, P, 2 * F8P * PAIR_W], mybir.dt.float8e4,
        kind="ExternalInput",
    )
    # Per-out-feature scales s[n] = mean_k |w[n, k]|, computed on host during
    # input prep (the sharding hint ships per-row scales with their rows) and
    # laid out per n-tile as a [128, 1] per-partition column (column i =
    # n-tile i).  Loading 2KB here instead of reducing 4MB of |w| on device
    # keeps the scale off the first eviction's critical path.
    sc = nc.dram_tensor("sc", [P, N_TILES], mybir.dt.float32, kind="ExternalInput")
    # w.T pre-tiled in k-tile pairs: wTt[jj, p, h*512+n] = w.T[(2*jj+h)*128+p, n]
    # so each DMA has 2KB-per-partition packets.
    wTt = nc.dram_tensor(
        "wTt", [K_TILES // 2, P, 2 * N_SHARD], mybir.dt.bfloat16,
        kind="ExternalInput",
    )
    outT = nc.dram_tensor(
        "outT", [N_SHARD, M_TOT], mybir.dt.float32, kind="ExternalOutput"
    )

    with tile.TileContext(nc) as tc, ExitStack() as ctx:
        spool = ctx.enter_context(tc.tile_pool(name="scales", bufs=1))
        wtpool = ctx.enter_context(tc.tile_pool(name="wtrans", bufs=6))
        sgpool = ctx.enter_context(tc.tile_pool(name="sign", bufs=1))
        sg8pool = ctx.enter_context(tc.tile_pool(name="sign8", bufs=1))
        xpool = ctx.enter_context(tc.tile_pool(name="xpair", bufs=2))
        x8pool = ctx.enter_context(tc.tile_pool(name="x8pair", bufs=2))
        opool = ctx.enter_context(tc.tile_pool(name="oblk", bufs=6))
        ppool = ctx.enter_context(tc.tile_pool(name="psum", bufs=8, space="PSUM"))

        # PE pre-warm: a few junk matmuls issued during the NEFF preamble
        # dead time keep the PE busy from ~7.4us so the HAM clock gate
        # reaches K=8/8 (2.4 GHz) before the first real matmul; otherwise
        # the first ~13 real matmuls run at 1.2 GHz (427ns cadence).
        dmy = spool.tile([P, 5 * P], mybir.dt.bfloat16, name="dmy")
        nc.gpsimd.memset(dmy[:], 0.0)
        pwarm = ppool.tile([P, M_BLK], mybir.dt.float32, tag="ps", name="ps_warm")
        for _ in range(9):
            nc.tensor.matmul(
                pwarm[:], dmy[:, 0:P], dmy[:, P : 5 * P], start=True, stop=True
            )

        # Queue assignment: sync = x loads + sign-weight loads (chained in
        # emission order so the FIFO queue is deterministic); scalar =
        # scales load, then output stores (which must wait on evictions
        # and would stall x loads).
        prev_sync_dma = [None]

        def sync_load(dst, src):
            dma = nc.sync.dma_start(dst, src)
            if prev_sync_dma[0] is not None:
                # add_dep_helper(waiter, dependency): this load is ordered
                # after the previous one on the sync queue.
                tile.add_dep_helper(
                    dma.ins, prev_sync_dma[0].ins, sync=False,
                    reason="sync DMA queue emission order",
                )
            prev_sync_dma[0] = dma
            return dma

        def issue_x_pair(q):
            # bf16 tile [P, K16, 1024]; fp8 tile [P, 2*F8P, 1024] where the
            # middle dim (2*jf + h) is the DoubleRow slot dim per pair.
            # One DMA per tile (partition-major source layout).
            xt = xpool.tile([P, K16, PAIR_W], mybir.dt.bfloat16, tag="xpair")
            x8t = x8pool.tile(
                [P, 2 * F8P, PAIR_W], mybir.dt.float8e4, tag="x8pair"
            )
            sync_load(xt[:, :, :], xTr[q - 1, :, :])
            sync_load(x8t[:, :, :], x8r[q - 1, :, :])
            return xt, x8t

        def mm16(pss, xt, b, ni, j):
            nc.tensor.matmul(
                pss[ni][:],
                S_all[:, j * N_SHARD + ni * P : j * N_SHARD + (ni + 1) * P],
                xt[:, j, b * M_BLK : (b + 1) * M_BLK],
                start=(j == 0),
                stop=False,
            )

        def mm8(pss, x8t, b, ni, jf):
            nc.tensor.matmul(
                pss[ni][:],
                S8_all[:, 2 * jf : 2 * jf + 2, ni * P : (ni + 1) * P],
                x8t[:, 2 * jf : 2 * jf + 2, b * M_BLK : (b + 1) * M_BLK],
                start=False,
                stop=(jf == F8P - 1),
                perf_mode=mybir.MatmulPerfMode.DoubleRow,
            )

        def evict_block(pss, mb):
            # Evictions alternate between the scalar and vector engines so
            # the per-block eviction chain (and the kernel tail) is half as
            # long. Stores ride the scalar queue; for the final block the
            # sync queue (drained of x loads by then) takes half the store
            # triggers so the tail isn't serialized on one engine.
            last = mb == M_BLKS - 1
            for ni in range(N_TILES):
                dst = outT[ni * P : (ni + 1) * P, mb * M_BLK : (mb + 1) * M_BLK]
                if last and ni == N_TILES - 1:
                    # The kernel-end fence waits on the very last store, so
                    # evict the final n-tile in two half-width mul+store
                    # pairs on separate queues: the last store starts one
                    # half-mul earlier and covers half the bytes.
                    half = M_BLK // 2
                    for h in range(2):
                        ot = opool.tile(
                            [P, half], mybir.dt.float32, tag="ot", name="ot"
                        )
                        nc.vector.tensor_scalar_mul(
                            ot[:], pss[ni][:, h * half : (h + 1) * half],
                            s_all[:, ni : ni + 1],
                        )
                        eng = nc.sync if h else nc.scalar
                        eng.dma_start(
                            outT[
                                ni * P : (ni + 1) * P,
                                mb * M_BLK + h * half : mb * M_BLK + (h + 1) * half,
                            ],
                            ot[:],
                        )
                    continue
                ot = opool.tile([P, M_BLK], mybir.dt.float32, tag="ot", name="ot")
                if ni % 2 == 0:
                    nc.scalar.mul(ot[:], pss[ni][:], s_all[:, ni : ni + 1])
                else:
                    nc.vector.tensor_scalar_mul(
                        ot[:], pss[ni][:], s_all[:, ni : ni + 1]
                    )
                if last and ni % 2 == 1:
                    nc.sync.dma_start(dst, ot[:])
                else:
                    nc.scalar.dma_start(dst, ot[:])

        # Prologue: interleave sign-weight loads with the first x pair's
        # loads on the chained sync queue so the earliest matmuls are fed in
        # lockstep with minimal latency.
        S_all = sgpool.tile([P, K16 * N_SHARD], mybir.dt.bfloat16)
        S8_all = sg8pool.tile([P, 2 * F8P, N_SHARD], mybir.dt.float8e4)
        xt0 = xpool.tile([P, K16, PAIR_W], mybir.dt.bfloat16, tag="xpair")
        x8t0 = x8pool.tile([P, 2 * F8P, PAIR_W], mybir.dt.float8e4, tag="x8pair")
        # Zero bias for the Sign activations as a plain SBUF tile (a float
        # bias would pull in a const-AP DRAM load during the preamble), and a
        # dummy 1-column sign to hoist the ACT LUT table load off the
        # critical path of the first real sign.
        zbias = spool.tile([P, 1], mybir.dt.float32)
        nc.gpsimd.memset(zbias[:], 0.0)
        nc.scalar.activation(
            S_all[:, 0:1], zbias[:], mybir.ActivationFunctionType.Sign,
            bias=zbias[:],
        )
        # fp8 DoubleRow pair sign: straight into the e4m3 tile (the scalar
        # engine converts on output; +-1 is exact in e4m3).  These loads are
        # interleaved among the bf16 pairs below so they are not stuck at the
        # tail of the startup DMA chain (their signs feed the pair-0 fp8
        # section at ~47us; at the chain tail they arrived several us late
        # and stalled the PE long enough to re-throttle HAM).
        def load_sign_fp8(jf):
            wt_t = wtpool.tile([P, 2 * N_SHARD], mybir.dt.bfloat16)
            dma = sync_load(wt_t[:], wTt[K16 // 2 + jf, :, :])
            nc.scalar.activation(
                S8_all[:, 2 * jf : 2 * jf + 2, :],
                wt_t[:],
                mybir.ActivationFunctionType.Sign,
                bias=zbias[:],
            )
            return dma

        # Scales: one 2KB load on the (otherwise idle) scalar dma queue.
        s_all = spool.tile([P, N_TILES], mybir.dt.float32)
        nc.scalar.dma_start(s_all[:], sc[:, :])

        fp8_pending = list(range(F8P))
        for jj in range(K16 // 2):
            wt_t = wtpool.tile([P, 2 * N_SHARD], mybir.dt.bfloat16)
            if jj == 0:
                # Split the first load/sign so the very first matmul (k-tile
                # 0, n-tile 0) is unblocked by a 32KB load + 128-col sign
                # instead of the full 512KB/1024-col pair, and slot the
                # first moving tile's load between the two sign chunks so
                # matmul #0's operands land back to back.
                sync_load(wt_t[:, 0:P], wTt[0, :, 0:P])
                nc.scalar.activation(
                    S_all[:, 0:P], wt_t[:, 0:P],
                    mybir.ActivationFunctionType.Sign, bias=zbias[:],
                )
                sync_load(xt0[:, 0, 0:M_BLK], xTp16[0, :, 0:M_BLK])
                sync_load(wt_t[:, P:], wTt[0, :, P:])
                nc.scalar.activation(
                    S_all[:, P : 2 * N_SHARD], wt_t[:, P:],
                    mybir.ActivationFunctionType.Sign, bias=zbias[:],
                )
                sync_load(xt0[:, 0, M_BLK:PAIR_W], xTp16[0, :, M_BLK:])
                sync_load(xt0[:, 1, :], xTp16[1, :, :])
            else:
                sync_load(wt_t[:], wTt[jj, :, :])
                nc.scalar.activation(
                    S_all[:, 2 * jj * N_SHARD : (2 * jj + 2) * N_SHARD],
                    wt_t[:],
                    mybir.ActivationFunctionType.Sign,
                    bias=zbias[:],
                )
                for j in (2 * jj, 2 * jj + 1):
                    sync_load(xt0[:, j, :], xTp16[j, :, :])
                if fp8_pending:
                    load_sign_fp8(fp8_pending.pop(0))
        while fp8_pending:
            load_sign_fp8(fp8_pending.pop(0))
        for jf in range(F8P):
            sync_load(x8t0[:, 2 * jf : 2 * jf + 2, :], x8p[jf, :, :])

        # Main loop: out.T[n, m] = sum_k S[k, n] * xT[k, m], scaled by s[n].
        # Pair 0 is computed j-outer across BOTH blocks (8 PSUM banks) so the
        # PE keeps pace with the HBM-limited startup stream; later pairs run
        # block-at-a-time j-outer (4 banks ping-ponging with the previous
        # block's draining 4).
        for q in range(M_PAIRS):
            xt, x8t = (xt0, x8t0) if q == 0 else issue_x_pair(q)
            if q == 0:
                pss2 = [
                    [
                        ppool.tile(
                            [P, M_BLK], mybir.dt.float32, tag="ps",
                            name=f"ps_{b}_{ni}",
                        )
                        for ni in range(N_TILES)
                    ]
                    for b in range(2)
                ]
                for j in range(K16):
                    for b in range(2):
                        for ni in range(N_TILES):
                            mm16(pss2[b], xt, b, ni, j)
                # fp8 tail of the warmup runs b-outer so block 0's stop
                # matmuls + eviction land ~7us early, freeing its PSUM banks
                # before pair 1's first accumulation needs them.
                for b in range(2):
                    for jf in range(F8P):
                        for ni in range(N_TILES):
                            mm8(pss2[b], x8t, b, ni, jf)
                    evict_block(pss2[b], b)
            else:
                for b in range(2):
                    last_blk = q == M_PAIRS - 1 and b == 1
                    pss = [
                        ppool.tile(
                            [P, M_BLK], mybir.dt.float32, tag="ps", name=f"ps{ni}"
                        )
                        for ni in range(N_TILES)
                    ]
                    if last_blk:
                        # ni-outer for the final block: each n-tile's stop
                        # matmul lands early, so its eviction + store overlap
                        # the remaining matmuls instead of serializing after
                        # the last one.
                        for ni in range(N_TILES):
                            for j in range(K16):
                                mm16(pss, xt, b, ni, j)
                            for jf in range(F8P):
                                mm8(pss, x8t, b, ni, jf)
                    else:
                        for j in range(K16):
                            for ni in range(N_TILES):
                                mm16(pss, xt, b, ni, j)
                        for jf in range(F8P):
                            for ni in range(N_TILES):
                                mm8(pss, x8t, b, ni, jf)
                    evict_block(pss, 2 * q + b)

    nc.compile()
    return nc


def kernel(x, weight):
    global LAST_RESULTS
    nc = _CACHE.get("nc")
    if nc is None:
        nc = _CACHE["nc"] = _build_nc()

    x = np.asarray(x)
    weight = np.asarray(weight)
    orig_shape = x.shape

    # Host-side sharding/layout: xT pre-tiled (replicated): bf16 k-tiles
    # [0, K16), e4m3 DoubleRow pairs [K16, 32); weight shard in both layouts.
    xT = x.reshape(M_TOT, D_IN).T  # [D_IN, M_TOT] view
    xTk = xT.reshape(K_TILES, P, M_PAIRS, PAIR_W)  # [kt, p, q, m]
    xb = xTk[:K16].astype(ml_dtypes.bfloat16)       # [kt, p, q, m]
    xf = xTk[K16:].astype(ml_dtypes.float8_e4m3)    # [kt'=2jf+h, p, q, m]
    # Pair 0 per-chunk layouts; pairs 1..7 partition-major (one DMA each).
    xTp16 = np.ascontiguousarray(xb[:, :, 0, :])    # [j, p, 1024]
    xTr = np.ascontiguousarray(
        xb.transpose(2, 1, 0, 3)[1:].reshape(M_PAIRS - 1, P, K16 * PAIR_W)
    )
    x8p = np.ascontiguousarray(
        xf[:, :, 0, :]
        .reshape(F8P, 2, P, PAIR_W)
        .transpose(0, 2, 1, 3)
        .reshape(F8P, P, 2 * PAIR_W)
    )  # [jf, p, h*1024+m]
    x8r = np.ascontiguousarray(
        xf.transpose(2, 1, 0, 3)[1:].reshape(M_PAIRS - 1, P, 2 * F8P * PAIR_W)
    )
    wt_full = np.ascontiguousarray(weight.T)  # [D_IN, D_OUT] f32
    # Per-row scales, sharded with their rows (see sharding hint): sc[p, i] =
    # mean|w[shard_base + i*128 + p, :]| as an f32 [128, N_TILES] column set.
    scales = np.abs(weight).mean(axis=1, dtype=np.float64).astype(np.float32)
    in_maps = []
    for c in range(N_CORES):
        in_maps.append(
            {
                "xTp16": xTp16,
                "xTr": xTr,
                "x8p": x8p,
                "x8r": x8r,
                "sc": np.ascontiguousarray(
                    scales[c * N_SHARD : (c + 1) * N_SHARD]
                    .reshape(N_TILES, P)
                    .T
                ),
                "wTt": np.ascontiguousarray(
                    wt_full[:, c * N_SHARD : (c + 1) * N_SHARD]
                    .reshape(K_TILES // 2, 2, P, N_SHARD)
                    .transpose(0, 2, 1, 3)
                    .reshape(K_TILES // 2, P, 2 * N_SHARD)
                    .astype(ml_dtypes.bfloat16)
                ),
            }
        )

    trace = bool(int(os.environ.get("BITLIN_TRACE", "0")))
    if trace:
        trace = _install_ntff_hook()
        base = os.environ.get("BITLIN_TRACE_DIR") or None
        if base:
            import tempfile

            os.makedirs(base, exist_ok=True)
            tmpdir = tempfile.mkdtemp(dir=base)
        else:
            tmpdir = None
    else:
        tmpdir = None
    res = bass_utils.run_bass_kernel_spmd(
        nc, in_maps, core_ids=list(range(N_CORES)), trace=trace, tmpdir=tmpdir
    )
    LAST_RESULTS = res

    outT_full = np.concatenate(
        [np.asarray(res.results[c]["outT"]) for c in range(N_CORES)], axis=0
    )  # [D_OUT, M_TOT] f32
    out = np.ascontiguousarray(outT_full.T).reshape(orig_shape).astype(np.float32)
    return out
